# revision 1
# baseline (speedup 1.0000x reference)
"""BoltzmannRouter Trainium2 kernel: 8-core data-parallel Bass implementation.

Full inputs: x (4, 4096, 2048) f32, gate_w (64, 2048) f32.
Output: routing weights (4, 4096, 64) f32 (softmax -> top-44 mask -> renorm).

Sharding: 16384 tokens split 2048/core across 8 NeuronCores; gate weight
replicated. Host pre-transposes each x shard to [D, tokens] so the device
DMA loads contraction-major tiles at full bandwidth, and pre-scales gate_w
by 1/TEMPERATURE.
"""

import os
import sys

sys.path.insert(0, "/opt/trn_rl_repo")

import numpy as np

D = 2048
E = 64
N_BOTTOM = 20  # 64 experts - 44 active
EPS = 1e-8
NEG_BIG = -1e30
TEMPERATURE = 2.718281828459045
N_CORES = 8
TPC = 2048  # tokens per core
GROUP = 512  # tokens per matmul group (one PSUM bank)

# Matmul input dtype: float32r streams fp32 at full rate (vs 4 cyc/row for
# plain float32) when the moving free dim >= 256; numerically identical fp32.
_MM_DTYPE_NAME = os.environ.get("BOLTZ_MM_DTYPE", "float32r")


def _build_nc():
    import concourse.bacc as bacc
    import concourse.mybir as mybir
    from concourse.masks import make_identity
    from concourse.tile import TileContext

    F32 = mybir.dt.float32
    mm_dtype = getattr(mybir.dt, _MM_DTYPE_NAME)
    kc_n = D // 128
    n_groups = TPC // GROUP
    n_sub = GROUP // 128

    nc = bacc.Bacc(None, target_bir_lowering=False)
    xT = nc.declare_dram_parameter("xT", [D, TPC], mm_dtype, isOutput=False)
    wT = nc.declare_dram_parameter("wT", [D, E], mm_dtype, isOutput=False)
    out = nc.declare_dram_parameter("out", [TPC, E], F32, isOutput=True)

    with TileContext(nc) as tc:
        with (
            tc.tile_pool(name="const", bufs=1) as cpool,
            tc.tile_pool(name="xg", bufs=2) as xpool,
            tc.tile_pool(name="sneg", bufs=2) as spool,
            tc.tile_pool(name="og", bufs=2) as opool,
            tc.tile_pool(name="work", bufs=3) as wkpool,
            tc.tile_pool(name="small", bufs=8) as smpool,
            tc.tile_pool(name="ps_s", bufs=2, space="PSUM") as ps_s_pool,
            tc.tile_pool(name="ps_t", bufs=4, space="PSUM") as ps_t_pool,
        ):
            ident = cpool.tile([E, E], F32)
            make_identity(nc, ident)

            w_sb = cpool.tile([128, kc_n, E], mm_dtype)
            nc.sync.dma_start(
                out=w_sb, in_=wT[:, :].rearrange("(kc p) e -> p kc e", p=128)
            )

            for g in range(n_groups):
                xg = xpool.tile([128, kc_n, GROUP], mm_dtype, tag="xg")
                for kc in range(kc_n):
                    nc.sync.dma_start(
                        out=xg[:, kc, :],
                        in_=xT[kc * 128 : (kc + 1) * 128, g * GROUP : (g + 1) * GROUP],
                    )

                # scores, expert-major: psum_s[e, t] = sum_d w[d,e] * x[d,t]
                psum_s = ps_s_pool.tile([E, GROUP], F32, tag="ps_s")
                for kc in range(kc_n):
                    nc.tensor.matmul(
                        psum_s,
                        lhsT=w_sb[:, kc, :],
                        rhs=xg[:, kc, :],
                        start=(kc == 0),
                        stop=(kc == kc_n - 1),
                    )

                # negate while copying PSUM->SBUF: sneg = -scores
                sneg = spool.tile([E, GROUP], F32, tag="sneg")
                nc.scalar.mul(sneg, psum_s, -1.0)

                og = opool.tile([128, n_sub, E], F32, tag="og")

                for s in range(n_sub):
                    # token-major negated scores [128 tok, 64 e]
                    psum_t = ps_t_pool.tile([128, E], F32, tag="ps_t")
                    nc.tensor.transpose(
                        psum_t, sneg[:, s * 128 : (s + 1) * 128], ident
                    )

                    # exp bias: min(-scores) = -max(scores)
                    mn = smpool.tile([128, 1], F32, tag="mn")
                    nc.vector.tensor_reduce(
                        mn, psum_t, axis=mybir.AxisListType.X, op=mybir.AluOpType.min
                    )
                    # u = exp(scores - max); S = sum(u)
                    u = wkpool.tile([128, E], F32, tag="u")
                    S = smpool.tile([128, 1], F32, tag="S")
                    nc.scalar.activation(
                        u,
                        psum_t,
                        mybir.ActivationFunctionType.Exp,
                        bias=mn,
                        scale=-1.0,
                        accum_out=S,
                    )

                    # threshold = 21st smallest score (negated domain: top-8
                    # of -scores are the smallest scores; 2x8 removed, then
                    # rank 17-24 -> index 4 = 21st)
                    y = wkpool.tile([128, E], F32, tag="y")
                    nc.vector.tensor_copy(y, psum_t)
                    r1 = smpool.tile([128, 8], F32, tag="r1")
                    nc.vector.max(r1, y)
                    nc.vector.match_replace(y, r1, y, NEG_BIG)
                    r2 = smpool.tile([128, 8], F32, tag="r2")
                    nc.vector.max(r2, y)
                    nc.vector.match_replace(y, r2, y, NEG_BIG)
                    r3 = smpool.tile([128, 8], F32, tag="r3")
                    nc.vector.max(r3, y)
                    thr = r3[:, (N_BOTTOM - 16) : (N_BOTTOM - 16 + 1)]

                    # wm = u * (-scores <= thr); ws = sum(wm)
                    wm = wkpool.tile([128, E], F32, tag="wm")
                    ws = smpool.tile([128, 1], F32, tag="ws")
                    nc.vector.scalar_tensor_tensor(
                        out=wm,
                        in0=psum_t,
                        scalar=thr,
                        in1=u,
                        op0=mybir.AluOpType.is_le,
                        op1=mybir.AluOpType.mult,
                        accum_out=ws,
                    )
                    # den = S*eps + ws; out = wm * (1/den)
                    den = smpool.tile([128, 1], F32, tag="den")
                    nc.vector.scalar_tensor_tensor(
                        out=den,
                        in0=S,
                        scalar=EPS,
                        in1=ws,
                        op0=mybir.AluOpType.mult,
                        op1=mybir.AluOpType.add,
                    )
                    rd = smpool.tile([128, 1], F32, tag="rd")
                    nc.vector.reciprocal(rd, den)
                    nc.vector.tensor_scalar_mul(og[:, s, :], wm, rd)

                nc.sync.dma_start(
                    out=out[g * GROUP : (g + 1) * GROUP, :].rearrange(
                        "(s p) e -> p s e", p=128
                    ),
                    in_=og,
                )

    nc.finalize()
    return nc


_NC = None
LAST_EXEC_NS = None
LAST_RESULTS = None


def _get_nc():
    global _NC
    if _NC is None:
        _NC = _build_nc()
    return _NC


def kernel(x, gate_w, trace=False):
    global LAST_EXEC_NS, LAST_RESULTS
    from concourse.bass_utils import run_bass_kernel_spmd

    x = np.asarray(x)
    gate_w = np.asarray(gate_w)
    Btot = x.shape[0] * x.shape[1]
    x2 = np.ascontiguousarray(x.reshape(Btot, D).astype(np.float32, copy=False))
    wTs = np.ascontiguousarray(
        gate_w.astype(np.float32, copy=False).T / np.float32(TEMPERATURE)
    )

    nc = _get_nc()
    in_maps = []
    for i in range(N_CORES):
        shard = np.ascontiguousarray(x2[i * TPC : (i + 1) * TPC].T)
        in_maps.append({"xT": shard, "wT": wTs})

    kwargs = {}
    if trace:
        kwargs["trace"] = True
    res = run_bass_kernel_spmd(nc, in_maps, core_ids=list(range(N_CORES)), **kwargs)
    LAST_EXEC_NS = res.exec_time_ns
    LAST_RESULTS = res
    out = np.concatenate([res.results[i]["out"] for i in range(N_CORES)], axis=0)
    return out.reshape(x.shape[0], x.shape[1], E)
